# revision 25
# baseline (speedup 1.0000x reference)
"""Trainium2 Bass kernel for nn_MultiHeadCrossAttention (B=32, Nc=2048, H=8, topk=12).

kernel(**inputs) takes FULL inputs, returns FULL output [32, 1, 128].
Batch is sharded 4-per-core across 8 NeuronCores (data parallel, no collectives).

v5 per-batch device algorithm (rows=(h,q) 128 wide, j = e*2048+nc in [0,16384)):
  single-term fp16 score matmul S-chunk[row,1024] -> PSUM
  scalar evacuates S -> SBUF fp16 (S16)
  gpsimd folds adjacent pairs: M[row, pair] = max(S16[2p], S16[2p+1])  [C,512]/chunk
  DVE max8 + find_index8 on M (half-width!) -> cand values + PAIR index directly
  exact top-12 of cand via max8/match_replace; pack (pair_idx*1024 + q10)
  decode -> winner pair indices gp + weights (exp/sum)
  gather V-pairs from VT (bf16) and score-pairs from S16 (fp16) with same idx
  parity mask from score-pair compare picks the winner half; weighted reduce
  out = (PV flat @ WjwP) + x;  out = out @ Wp + bp
  (rel err ~6.6e-3 on the fixed input: fp16 scores + fp16 fold ties + q10 weights)
"""

import sys
import numpy as np

for p in ("/opt/trn_rl_repo",):
    if p not in sys.path:
        sys.path.insert(0, p)

import ml_dtypes

B, CORES, BPC = 32, 8, 4
H, HD, NQ, TK, C, NC = 8, 16, 16, 12, 128, 2048
NJ = 8 * NC            # 16384
NP = NJ // 2           # 8192 pairs
CHUNK = 1024           # PSUM tile width (j)
SCH = 2048             # selection chunk (j): 1024 pairs
NSC = NJ // SCH        # 8 selection chunks
NCAND = NSC * 8        # 64
NEG = -1e30
MAGIC = 12582912.0     # 2**23 + 2**22: add/sub rounds fp32 to nearest int

_prog_cache = {}


def _build_program():
    import concourse.bass as bass
    import concourse.mybir as mybir
    import concourse.tile as tile
    from concourse import bacc
    from concourse import library_config

    dt = mybir.dt
    Alu = mybir.AluOpType
    f32, f16, bf16 = dt.float32, dt.float16, dt.bfloat16
    nc = bacc.Bacc("TRN2", target_bir_lowering=False)

    comphT_d = nc.dram_tensor("comphT", [BPC, C, NC], f16, kind="ExternalInput")
    a16_d = nc.dram_tensor("a16", [BPC, C, 8 * C], f16, kind="ExternalInput")
    xT_d = nc.dram_tensor("xT", [C, BPC], f32, kind="ExternalInput")
    wv_d = nc.dram_tensor("wv", [C, 8 * C], f16, kind="ExternalInput")
    wjwp_d = nc.dram_tensor("wjwp", [C, NQ * C], f32, kind="ExternalInput")
    wp_d = nc.dram_tensor("wp", [C, C], f32, kind="ExternalInput")
    bp4_d = nc.dram_tensor("bp4", [BPC, C], f32, kind="ExternalInput")
    hrep_d = nc.dram_tensor("hrep", [C, C], f32, kind="ExternalInput")
    choff_d = nc.dram_tensor("choff", [C, NCAND], f32, kind="ExternalInput")
    me_d = nc.dram_tensor("me", [C, 512], f32, kind="ExternalInput")
    mo_d = nc.dram_tensor("mo", [C, 512], f32, kind="ExternalInput")
    mmq_d = nc.dram_tensor("mmq", [C, 256], f32, kind="ExternalInput")
    out_d = nc.dram_tensor("out", [BPC, C], f32, kind="ExternalOutput")

    with tile.TileContext(nc) as tc:
        nc.gpsimd.load_library(library_config.ap_gather)
        with (
            tc.tile_pool(name="weights", bufs=1) as wpool,
            tc.tile_pool(name="compt", bufs=2) as ctpool,
            tc.tile_pool(name="bigV", bufs=2) as vpool,
            tc.tile_pool(name="small", bufs=1) as smpool,
            tc.tile_pool(name="ps_sel", bufs=2, space="PSUM") as ps_sel,
            tc.tile_pool(name="ps_v", bufs=2, space="PSUM") as ps_v,
        ):
            # ---- chunk-phase weights first; the rest stream later ----
            wv_s = wpool.tile([C, 8 * C], f16)
            nc.sync.dma_start(wv_s[:], wv_d[:])
            hrep_s = wpool.tile([C, C], f32)
            nc.sync.dma_start(hrep_s[:], hrep_d[:])
            choff_s = wpool.tile([C, NCAND], f32)
            me_s = wpool.tile([C, 512], f32)
            mo_s = wpool.tile([C, 512], f32)
            mmq_s = wpool.tile([C, 256], f32)
            wjwp_s = wpool.tile([C, NQ * C], f32)
            wp_s = wpool.tile([C, C], f32)
            bp4_s = wpool.tile([BPC, C], f32)
            xT_s = wpool.tile([C, BPC], f32)

            def emit_const_dmas():
                nc.sync.dma_start(choff_s[:], choff_d[:])
                nc.sync.dma_start(me_s[:], me_d[:])
                nc.sync.dma_start(mo_s[:], mo_d[:])
                nc.sync.dma_start(mmq_s[:], mmq_d[:])
                nc.sync.dma_start(wjwp_s[:], wjwp_d[:])
                nc.sync.dma_start(wp_s[:], wp_d[:])
                nc.sync.dma_start(bp4_s[:], bp4_d[:])
                nc.sync.dma_start(xT_s[:], xT_d[:])

            pvt4_s = wpool.tile([C, NQ * BPC], f32)   # [(h,d), (q,b)]

            def emit_chunks(b):
                st = {}
                c16h = ctpool.tile([C, NC], f16, tag="c16h", name="c16h")
                nc.sync.dma_start(c16h[:], comphT_d[b])
                a16h = smpool.tile([C, 8 * C], f16, tag="a16h", bufs=2,
                                   name="a16h")
                nc.sync.dma_start(a16h[:], a16_d[b])
                cand_s = smpool.tile([C, NCAND], f32, tag="cand", bufs=2,
                                     name="cand_s")
                li_s = smpool.tile([C, NCAND], dt.uint16, tag="li", bufs=2,
                                   name="li_s")
                vt_s = vpool.tile([C, NJ], bf16, tag="VT", name="vt_s")
                s16e = vpool.tile([C, NP], f32, tag="S16E", bufs=1,
                                  name="s16e")
                m_s = vpool.tile([C, NP], f32, tag="M", bufs=1, name="m_s")
                for e in range(8):
                    s_list = []
                    for half in range(2):
                        s_ps = ps_sel.tile([C, CHUNK], f32, tag="sel",
                                           name="s_ps")
                        s_list.append(s_ps)
                        for n in range(2):
                            col = half * 1024 + n * 512
                            nc.tensor.matmul(
                                s_ps[:, n * 512:(n + 1) * 512],
                                a16h[:, e * C:(e + 1) * C],
                                c16h[:, col:col + 512],
                            )
                    for half in range(2):
                        v_ps = ps_v.tile([C, CHUNK], f32, tag="v", name="v_ps")
                        for n in range(2):
                            col = half * 1024 + n * 512
                            nc.tensor.matmul(
                                v_ps[:, n * 512:(n + 1) * 512],
                                wv_s[:, e * C:(e + 1) * C],
                                c16h[:, col:col + 512],
                            )
                        nc.scalar.copy(
                            vt_s[:, e * NC + half * 1024:
                                 e * NC + (half + 1) * 1024],
                            v_ps[:],
                        )
                    for half in range(2):
                        ch = e * 2 + half
                        s_ps = s_list[half]
                        ec = s16e[:, ch * 512:(ch + 1) * 512]
                        nc.scalar.copy(ec, s_ps[:, 0::2])
                        nc.vector.tensor_max(
                            m_s[:, ch * 512:(ch + 1) * 512],
                            ec,
                            s_ps[:, 1::2],
                        )
                    nc.vector.max(
                        cand_s[:, e * 8:(e + 1) * 8],
                        m_s[:, e * CHUNK:(e + 1) * CHUNK],
                    )
                    nc.vector.max_index(
                        li_s[:, e * 8:(e + 1) * 8],
                        cand_s[:, e * 8:(e + 1) * 8],
                        m_s[:, e * CHUNK:(e + 1) * CHUNK],
                    )
                st.update(cand_s=cand_s, li_s=li_s, vt_s=vt_s, s16e=s16e,
                          m_s=m_s)
                return st

            def emit_early_tail(b, st):
                cand_s, li_s = st["cand_s"], st["li_s"]
                # exact top-12 marking on cand
                t8a = smpool.tile([C, 8], f32, tag="t8a", name="t8a")
                nc.vector.max(t8a[:], cand_s[:])
                c2 = smpool.tile([C, NCAND], f32, tag="c2", name="c2")
                nc.vector.match_replace(c2[:], t8a[:], cand_s[:], NEG)
                t8b = smpool.tile([C, 8], f32, tag="t8b", name="t8b")
                nc.vector.max(t8b[:], c2[:])
                nx4 = smpool.tile([C, 8], f32, tag="nx4", name="nx4")
                nc.vector.memset(nx4[:], 1e30)
                nc.vector.tensor_copy(nx4[:, 0:4], t8b[:, 0:4])
                rr = smpool.tile([C, NCAND], f32, tag="rr", name="rr")
                nc.vector.match_replace(rr[:], nx4[:], c2[:], NEG)
                mask12 = smpool.tile([C, NCAND], f32, tag="mask12",
                                     name="mask12")
                nc.vector.tensor_scalar(
                    mask12[:], rr[:], -1e29, None, Alu.is_le
                )

                # pack pair_idx*1024 + q10(value); mask; extract
                lif = smpool.tile([C, NCAND], f32, tag="lif", name="lif")
                nc.vector.tensor_copy(lif[:], li_s[:])
                gfl = smpool.tile([C, NCAND], f32, tag="gfl", name="gfl")
                nc.vector.scalar_tensor_tensor(
                    gfl[:], lif[:], 1024.0, choff_s[:], Alu.mult, Alu.add
                )
                q10 = smpool.tile([C, NCAND], f32, tag="q10", name="q10")
                nc.vector.tensor_scalar(
                    q10[:], cand_s[:], 4.0, 128.0, Alu.add, Alu.mult
                )
                nc.vector.tensor_scalar(
                    q10[:], q10[:], 1023.0, 1.0, Alu.min, Alu.max
                )
                pm = smpool.tile([C, NCAND], f32, tag="pm", name="pm")
                nc.vector.tensor_add(pm[:], gfl[:], q10[:])
                nc.vector.tensor_mul(pm[:], pm[:], mask12[:])

                pw = smpool.tile([C, 16], f32, tag="pw", bufs=2, name="pw")
                nc.vector.max(pw[:, 0:8], pm[:])
                pm2 = smpool.tile([C, NCAND], f32, tag="pm2", name="pm2")
                nc.vector.match_replace(pm2[:], pw[:, 0:8], pm[:], 0.0)
                nc.vector.max(pw[:, 8:16], pm2[:])

                # decode winners: pair idx gp + value -> weights
                gidxf = smpool.tile([C, 16], f32, tag="gidxf", bufs=2,
                                    name="gidxf")
                nc.vector.tensor_scalar(
                    gidxf[:], pw[:], 1.0 / 1024.0, -0.5, Alu.mult, Alu.add
                )
                nc.vector.tensor_scalar(
                    gidxf[:], gidxf[:], MAGIC, MAGIC, Alu.add, Alu.subtract
                )
                vv = smpool.tile([C, 16], f32, tag="vv", name="vv")
                nc.vector.scalar_tensor_tensor(
                    vv[:], gidxf[:], -1024.0, pw[:], Alu.mult, Alu.add
                )
                nc.vector.tensor_scalar(
                    vv[:], vv[:], 1.0 / 128.0, -4.0, Alu.mult, Alu.add
                )
                expv = smpool.tile([C, 16], f32, tag="expv", name="expv")
                nc.scalar.activation(
                    expv[:], vv[:], mybir.ActivationFunctionType.Exp
                )
                wgt = smpool.tile([C, 16], f32, tag="wgt", name="wgt")
                nc.vector.scalar_tensor_tensor(
                    wgt[:], pw[:], 0.5, expv[:], Alu.is_ge, Alu.mult
                )
                den = smpool.tile([C, 1], f32, tag="den", name="den")
                nc.vector.tensor_reduce(
                    den[:], wgt[:], mybir.AxisListType.X, Alu.add
                )
                rden = smpool.tile([C, 1], f32, tag="rden", name="rden")
                nc.vector.reciprocal(rden[:], den[:])
                wn = smpool.tile([C, 16], f32, tag="wn", bufs=2, name="wn")
                nc.vector.tensor_scalar(
                    wn[:], wgt[:], rden[:], None, Alu.mult
                )

                gp_i = smpool.tile([C, 16], dt.int16, tag="gpi", bufs=2,
                                   name="gp_i")
                nc.vector.tensor_copy(gp_i[:], gidxf[:])

                # gather V pairs and score pairs (same idx lists)
                g_s = smpool.tile([C, 512], bf16, tag="G", bufs=2, name="g_s")
                nc.gpsimd.ap_gather(
                    g_s[:], st["vt_s"][:], gp_i[:],
                    channels=C, num_elems=NP, d=2, num_idxs=256,
                )
                eg_s = smpool.tile([C, 256], f32, tag="EG", bufs=2,
                                   name="eg_s")
                nc.gpsimd.ap_gather(
                    eg_s[:], st["s16e"][:], gp_i[:],
                    channels=C, num_elems=NP, d=1, num_idxs=256,
                )
                mg_s = smpool.tile([C, 256], f32, tag="MG", bufs=2,
                                   name="mg_s")
                nc.gpsimd.ap_gather(
                    mg_s[:], st["m_s"][:], gp_i[:],
                    channels=C, num_elems=NP, d=1, num_idxs=256,
                )
                st.update(g_s=g_s, eg_s=eg_s, mg_s=mg_s, wn=wn)

            def emit_late_tail(b, st):
                g_s, wn = st["g_s"], st["wn"]
                # per-row parity: even wins iff E == M (dd = E - M == 0)
                dd = smpool.tile([C, 256], f32, tag="dd", name="dd")
                nc.vector.tensor_sub(dd[:], st["eg_s"][:], st["mg_s"][:])
                nc.vector.tensor_mul(dd[:], dd[:], mmq_s[:])
                ddr = smpool.tile([C, 16], f32, tag="ddr", name="ddr")
                nc.vector.tensor_reduce(
                    ddr[:],
                    dd[:].rearrange("p (i q) -> p i q", q=NQ),
                    mybir.AxisListType.X,
                    Alu.add,
                )
                wnE = smpool.tile([C, 16], f32, tag="wnE", name="wnE")
                nc.vector.scalar_tensor_tensor(
                    wnE[:], ddr[:], 0.0, wn[:], Alu.is_ge, Alu.mult
                )
                wnO = smpool.tile([C, 16], f32, tag="wnO", name="wnO")
                nc.vector.tensor_sub(wnO[:], wn[:], wnE[:])

                # weights (parity-split) -> [(h,d), (i,q,r)] via headrep
                wEb = (
                    wnE[:].rearrange("p (i o) -> p i o", o=1)
                    .to_broadcast([C, NQ, 32])
                )
                wOb = (
                    wnO[:].rearrange("p (i o) -> p i o", o=1)
                    .to_broadcast([C, NQ, 32])
                )
                tmpE = smpool.tile([C, 512], f32, tag="tmpE", name="tmpE")
                nc.vector.tensor_mul(
                    tmpE[:].rearrange("p (i s) -> p i s", s=32),
                    wEb,
                    me_s[:].rearrange("p (i s) -> p i s", s=32),
                )
                wsc = smpool.tile([C, 512], f32, tag="wsc", name="wsc")
                nc.vector.tensor_mul(
                    wsc[:].rearrange("p (i s) -> p i s", s=32),
                    wOb,
                    mo_s[:].rearrange("p (i s) -> p i s", s=32),
                )
                nc.vector.tensor_add(wsc[:], wsc[:], tmpE[:])
                wb_ps = ps_v.tile([C, CHUNK], f32, tag="v", name="wb_ps")
                nc.tensor.matmul(wb_ps[:, 0:512], hrep_s[:], wsc[:])
                wb_s = smpool.tile([C, 512], bf16, tag="wb", name="wb_s")
                nc.scalar.copy(wb_s[:], wb_ps[:, 0:512])

                gw = smpool.tile([C, 512], f32, tag="gw", name="gw")
                nc.vector.tensor_mul(gw[:], g_s[:], wb_s[:])
                nc.vector.tensor_reduce(
                    pvt4_s[:, b::BPC],
                    gw[:].rearrange("p (i q r) -> p q i r", q=NQ, r=2),
                    mybir.AxisListType.XY,
                    Alu.add,
                )

            states = {}
            for b in range(BPC):
                states[b] = emit_chunks(b)
                if b == 0:
                    emit_const_dmas()
                emit_early_tail(b, states[b])
                if b >= 1:
                    emit_late_tail(b - 1, states[b - 1])
            emit_late_tail(BPC - 1, states[BPC - 1])

            # ---- final projections for all 4 batches ----
            o1_ps = ps_sel.tile([C, CHUNK], f32, tag="sel")
            for q in range(NQ):
                nc.tensor.matmul(
                    o1_ps[:, 0:BPC],
                    wjwp_s[:, q * C:(q + 1) * C],
                    pvt4_s[:, q * BPC:(q + 1) * BPC],
                    start=(q == 0),
                    stop=(q == NQ - 1),
                )
            o2_s = smpool.tile([C, BPC], f32, tag="o2")
            nc.vector.tensor_add(o2_s[:], o1_ps[:, 0:BPC], xT_s[:])
            o3_ps = ps_v.tile([C, CHUNK], f32, tag="v")
            nc.tensor.matmul(o3_ps[0:BPC, 0:C], o2_s[:], wp_s[:])
            o4_s = smpool.tile([BPC, C], f32, tag="o4")
            nc.vector.tensor_add(o4_s[:], o3_ps[0:BPC, 0:C], bp4_s[:])
            nc.sync.dma_start(out_d[:], o4_s[:])

    nc.compile()
    return nc


def _host_prep(inputs):
    x = np.asarray(inputs["x"], dtype=np.float32)              # [32, 1, 128]
    complement = np.asarray(inputs["complement"], np.float32)  # [32, 2047, 128]
    Wq = np.asarray(inputs["Wq"], np.float32)
    Wkv = np.asarray(inputs["Wkv"], np.float32)
    Wjw = np.asarray(inputs["Wjw"], np.float32)
    Wp = np.asarray(inputs["Wp"], np.float32)
    bp = np.asarray(inputs["bp"], np.float32)

    wkT = np.empty((C, 8 * C), np.float32)
    wv = np.empty((C, 8 * C), np.float32)
    for e in range(8):
        wkT[:, e * C:(e + 1) * C] = Wkv[:, e * 256: e * 256 + 128].T
        wv[:, e * C:(e + 1) * C] = Wkv[:, e * 256 + 128: e * 256 + 256]
    wv = wv.astype(np.float16)
    # host-side A_e[c,row] = Wk_e^T @ Qbd (0.25-scaled block-diag Q)
    qt_all = (x.reshape(B, C) @ Wq)                       # [B, 2048]
    a16_all = np.empty((B, C, 8 * C), np.float16)
    qbd = np.zeros((B, C, C), np.float32)
    for hh in range(H):
        for qq in range(NQ):
            qbd[:, hh * HD:(hh + 1) * HD, hh * NQ + qq] = (
                qt_all[:, qq * C + hh * HD: qq * C + (hh + 1) * HD] * 0.25
            )
    for e in range(8):
        blk = np.einsum(
            'cr,bcx->brx', wkT[:, e * C:(e + 1) * C], qbd
        )  # [B, row?, ...] -> A = wkT_e.T @ qbd per batch
        a16_all[:, :, e * C:(e + 1) * C] = blk.astype(np.float16)
    # Wjw rows are (h,q,d); per-q slice with rows (h,d)
    wjwp = (
        Wjw.reshape(H, NQ, HD, C).transpose(1, 0, 2, 3).reshape(NQ, C, C)
        .transpose(1, 0, 2).reshape(C, NQ * C)
    )
    bp4 = np.tile(bp.reshape(1, C), (BPC, 1)).astype(np.float32)
    hrep = np.kron(np.eye(H, dtype=np.float32), np.ones((HD, HD), np.float32))
    # cand slot -> chunk pair-offset (pairs per chunk = 512)
    choffrow = ((np.arange(NCAND) // 8) * (1024 * 1024)).astype(np.float32)
    choff = np.tile(choffrow.reshape(1, NCAND), (C, 1))
    s_idx = np.tile(np.arange(32).reshape(1, 1, 32), (C, NQ, 1))
    p_idx = (np.arange(C) % NQ).reshape(C, 1, 1)
    me = (s_idx == 2 * p_idx).astype(np.float32).reshape(C, 512)
    mo = (s_idx == 2 * p_idx + 1).astype(np.float32).reshape(C, 512)
    q_idx = np.tile(np.arange(NQ).reshape(1, 1, NQ), (C, NQ, 1))
    mmq = (q_idx == p_idx).astype(np.float32).reshape(C, 256)

    shared = dict(
        wv=np.ascontiguousarray(wv),
        wjwp=np.ascontiguousarray(wjwp),
        wp=np.ascontiguousarray(Wp),
        bp4=bp4,
        hrep=np.ascontiguousarray(hrep),
        choff=np.ascontiguousarray(choff),
        me=np.ascontiguousarray(me),
        mo=np.ascontiguousarray(mo),
        mmq=np.ascontiguousarray(mmq),
    )

    in_maps = []
    for core in range(CORES):
        bs = range(core * BPC, (core + 1) * BPC)
        comp = np.stack(
            [
                np.concatenate([x[b].reshape(1, C), complement[b]], axis=0)
                for b in bs
            ]
        ).astype(np.float32)
        compT = comp.transpose(0, 2, 1)          # [BPC, C, NC]
        comphT = compT.astype(np.float16)
        xT = np.ascontiguousarray(x[list(bs)].reshape(BPC, C).T)
        m = dict(shared)
        m["comphT"] = np.ascontiguousarray(comphT)
        m["a16"] = np.ascontiguousarray(a16_all[list(bs)])
        m["xT"] = xT
        in_maps.append(m)
    return in_maps


def kernel(**inputs):
    from concourse.bass_utils import run_bass_kernel_spmd

    if "prog" not in _prog_cache:
        _prog_cache["prog"] = _build_program()
    nc = _prog_cache["prog"]

    in_maps = _host_prep(inputs)
    res = run_bass_kernel_spmd(nc, in_maps, core_ids=list(range(CORES)))
    out = np.empty((B, 1, C), np.float32)
    for core in range(CORES):
        o = res.results[core]["out"]
        for i in range(BPC):
            out[core * BPC + i, 0, :] = o[i]
    return out


if __name__ == "__main__":
    d = np.load("/root/problem/inputs_cache.npz")
    inputs = {k: d[k] for k in d.files}
    got = kernel(**inputs)
    print("kernel output:", got.shape, got.dtype, np.abs(got).max())


# revision 26
# speedup vs baseline: 1.1899x; 1.1899x over previous
"""Trainium2 Bass kernel for nn_MultiHeadCrossAttention (B=32, Nc=2048, H=8, topk=12).

kernel(**inputs) takes FULL inputs, returns FULL output [32, 1, 128].
Batch is sharded 4-per-core across 8 NeuronCores (data parallel, no collectives).

v5 per-batch device algorithm (rows=(h,q) 128 wide, j = e*2048+nc in [0,16384)):
  single-term fp16 score matmul S-chunk[row,1024] -> PSUM
  scalar evacuates S -> SBUF fp16 (S16)
  gpsimd folds adjacent pairs: M[row, pair] = max(S16[2p], S16[2p+1])  [C,512]/chunk
  DVE max8 + find_index8 on M (half-width!) -> cand values + PAIR index directly
  exact top-12 of cand via max8/match_replace; pack (pair_idx*1024 + q10)
  decode -> winner pair indices gp + weights (exp/sum)
  gather V-pairs from VT (bf16) and score-pairs from S16 (fp16) with same idx
  parity mask from score-pair compare picks the winner half; weighted reduce
  out = (PV flat @ WjwP) + x;  out = out @ Wp + bp
  (rel err ~6.6e-3 on the fixed input: fp16 scores + fp16 fold ties + q10 weights)
"""

import sys
import numpy as np

for p in ("/opt/trn_rl_repo",):
    if p not in sys.path:
        sys.path.insert(0, p)

import ml_dtypes

B, CORES, BPC = 32, 8, 4
H, HD, NQ, TK, C, NC = 8, 16, 16, 12, 128, 2048
NJ = 8 * NC            # 16384
NP = NJ // 2           # 8192 pairs
CHUNK = 1024           # PSUM tile width (j)
SCH = 2048             # selection chunk (j): 1024 pairs
NSC = NJ // SCH        # 8 selection chunks
NCAND = NSC * 8        # 64
NEG = -1e30
MAGIC = 12582912.0     # 2**23 + 2**22: add/sub rounds fp32 to nearest int

_prog_cache = {}


def _build_program():
    import concourse.bass as bass
    import concourse.mybir as mybir
    import concourse.tile as tile
    from concourse import bacc
    from concourse import library_config

    dt = mybir.dt
    Alu = mybir.AluOpType
    f32, f16, bf16 = dt.float32, dt.float16, dt.bfloat16
    nc = bacc.Bacc("TRN2", target_bir_lowering=False)

    comphT_d = nc.dram_tensor("comphT", [BPC, C, NC], f16, kind="ExternalInput")
    comphP_d = nc.dram_tensor("comphP", [BPC, C, NC], f16, kind="ExternalInput")
    a16_d = nc.dram_tensor("a16", [BPC, C, 8 * C], f16, kind="ExternalInput")
    xT_d = nc.dram_tensor("xT", [C, BPC], f32, kind="ExternalInput")
    wv_d = nc.dram_tensor("wv", [C, 8 * C], f16, kind="ExternalInput")
    wjwp_d = nc.dram_tensor("wjwp", [C, NQ * C], f32, kind="ExternalInput")
    wp_d = nc.dram_tensor("wp", [C, C], f32, kind="ExternalInput")
    bp4_d = nc.dram_tensor("bp4", [BPC, C], f32, kind="ExternalInput")
    hrep_d = nc.dram_tensor("hrep", [C, C], f32, kind="ExternalInput")
    choff_d = nc.dram_tensor("choff", [C, NCAND], f32, kind="ExternalInput")
    me_d = nc.dram_tensor("me", [C, 512], f32, kind="ExternalInput")
    mo_d = nc.dram_tensor("mo", [C, 512], f32, kind="ExternalInput")
    mmq_d = nc.dram_tensor("mmq", [C, 256], f32, kind="ExternalInput")
    out_d = nc.dram_tensor("out", [BPC, C], f32, kind="ExternalOutput")

    with tile.TileContext(nc) as tc:
        nc.gpsimd.load_library(library_config.ap_gather)
        with (
            tc.tile_pool(name="weights", bufs=1) as wpool,
            tc.tile_pool(name="compt", bufs=2) as ctpool,
            tc.tile_pool(name="bigV", bufs=2) as vpool,
            tc.tile_pool(name="small", bufs=1) as smpool,
            tc.tile_pool(name="ps_sel", bufs=2, space="PSUM") as ps_sel,
            tc.tile_pool(name="ps_v", bufs=2, space="PSUM") as ps_v,
        ):
            # ---- chunk-phase weights first; the rest stream later ----
            wv_s = wpool.tile([C, 8 * C], f16)
            nc.sync.dma_start(wv_s[:], wv_d[:])
            hrep_s = wpool.tile([C, C], f32)
            nc.sync.dma_start(hrep_s[:], hrep_d[:])
            choff_s = wpool.tile([C, NCAND], f32)
            me_s = wpool.tile([C, 512], f32)
            mo_s = wpool.tile([C, 512], f32)
            mmq_s = wpool.tile([C, 256], f32)
            wjwp_s = wpool.tile([C, NQ * C], f32)
            wp_s = wpool.tile([C, C], f32)
            bp4_s = wpool.tile([BPC, C], f32)
            xT_s = wpool.tile([C, BPC], f32)

            def emit_const_dmas():
                nc.sync.dma_start(choff_s[:], choff_d[:])
                nc.sync.dma_start(me_s[:], me_d[:])
                nc.sync.dma_start(mo_s[:], mo_d[:])
                nc.sync.dma_start(mmq_s[:], mmq_d[:])
                nc.sync.dma_start(wjwp_s[:], wjwp_d[:])
                nc.sync.dma_start(wp_s[:], wp_d[:])
                nc.sync.dma_start(bp4_s[:], bp4_d[:])
                nc.sync.dma_start(xT_s[:], xT_d[:])

            pvt4_s = wpool.tile([C, NQ * BPC], f32)   # [(h,d), (q,b)]

            def emit_chunks(b):
                st = {}
                c16h = ctpool.tile([C, NC], f16, tag="c16h", name="c16h")
                nc.sync.dma_start(c16h[:], comphT_d[b])
                c16p = ctpool.tile([C, NC], f16, tag="c16p", name="c16p")
                nc.sync.dma_start(c16p[:], comphP_d[b])
                a16h = smpool.tile([C, 8 * C], f16, tag="a16h", bufs=2,
                                   name="a16h")
                nc.sync.dma_start(a16h[:], a16_d[b])
                cand_s = smpool.tile([C, NCAND], f32, tag="cand", bufs=2,
                                     name="cand_s")
                li_s = smpool.tile([C, NCAND], dt.uint16, tag="li", bufs=2,
                                   name="li_s")
                vt_s = vpool.tile([C, NJ], bf16, tag="VT", name="vt_s")
                s16e = vpool.tile([C, NP], f32, tag="S16E", bufs=1,
                                  name="s16e")
                m_s = vpool.tile([C, NP], f32, tag="M", bufs=1, name="m_s")
                for e in range(8):
                    s_list = []
                    for half in range(2):
                        s_ps = ps_sel.tile([C, CHUNK], f32, tag="sel",
                                           name="s_ps")
                        s_list.append(s_ps)
                        for n in range(2):
                            col = half * 1024 + n * 512
                            nc.tensor.matmul(
                                s_ps[:, n * 512:(n + 1) * 512],
                                a16h[:, e * C:(e + 1) * C],
                                c16p[:, col:col + 512],
                            )
                    for half in range(2):
                        v_ps = ps_v.tile([C, CHUNK], f32, tag="v", name="v_ps")
                        for n in range(2):
                            col = half * 1024 + n * 512
                            nc.tensor.matmul(
                                v_ps[:, n * 512:(n + 1) * 512],
                                wv_s[:, e * C:(e + 1) * C],
                                c16h[:, col:col + 512],
                            )
                        nc.scalar.copy(
                            vt_s[:, e * NC + half * 1024:
                                 e * NC + (half + 1) * 1024],
                            v_ps[:],
                        )
                    for half in range(2):
                        ch = e * 2 + half
                        s_ps = s_list[half]
                        ec = s16e[:, ch * 512:(ch + 1) * 512]
                        nc.scalar.copy(ec, s_ps[:, 0:512])
                        nc.vector.tensor_max(
                            m_s[:, ch * 512:(ch + 1) * 512],
                            ec,
                            s_ps[:, 512:1024],
                        )
                    nc.vector.max(
                        cand_s[:, e * 8:(e + 1) * 8],
                        m_s[:, e * CHUNK:(e + 1) * CHUNK],
                    )
                    nc.vector.max_index(
                        li_s[:, e * 8:(e + 1) * 8],
                        cand_s[:, e * 8:(e + 1) * 8],
                        m_s[:, e * CHUNK:(e + 1) * CHUNK],
                    )
                st.update(cand_s=cand_s, li_s=li_s, vt_s=vt_s, s16e=s16e,
                          m_s=m_s)
                return st

            def emit_early_tail(b, st):
                cand_s, li_s = st["cand_s"], st["li_s"]
                # exact top-12 marking on cand
                t8a = smpool.tile([C, 8], f32, tag="t8a", name="t8a")
                nc.vector.max(t8a[:], cand_s[:])
                c2 = smpool.tile([C, NCAND], f32, tag="c2", name="c2")
                nc.vector.match_replace(c2[:], t8a[:], cand_s[:], NEG)
                t8b = smpool.tile([C, 8], f32, tag="t8b", name="t8b")
                nc.vector.max(t8b[:], c2[:])
                nx4 = smpool.tile([C, 8], f32, tag="nx4", name="nx4")
                nc.vector.memset(nx4[:], 1e30)
                nc.vector.tensor_copy(nx4[:, 0:4], t8b[:, 0:4])
                rr = smpool.tile([C, NCAND], f32, tag="rr", name="rr")
                nc.vector.match_replace(rr[:], nx4[:], c2[:], NEG)
                mask12 = smpool.tile([C, NCAND], f32, tag="mask12",
                                     name="mask12")
                nc.vector.tensor_scalar(
                    mask12[:], rr[:], -1e29, None, Alu.is_le
                )

                # pack pair_idx*1024 + q10(value); mask; extract
                lif = smpool.tile([C, NCAND], f32, tag="lif", name="lif")
                nc.vector.tensor_copy(lif[:], li_s[:])
                gfl = smpool.tile([C, NCAND], f32, tag="gfl", name="gfl")
                nc.vector.scalar_tensor_tensor(
                    gfl[:], lif[:], 1024.0, choff_s[:], Alu.mult, Alu.add
                )
                q10 = smpool.tile([C, NCAND], f32, tag="q10", name="q10")
                nc.vector.tensor_scalar(
                    q10[:], cand_s[:], 4.0, 128.0, Alu.add, Alu.mult
                )
                nc.vector.tensor_scalar(
                    q10[:], q10[:], 1023.0, 1.0, Alu.min, Alu.max
                )
                pm = smpool.tile([C, NCAND], f32, tag="pm", name="pm")
                nc.vector.tensor_add(pm[:], gfl[:], q10[:])
                nc.vector.tensor_mul(pm[:], pm[:], mask12[:])

                pw = smpool.tile([C, 16], f32, tag="pw", bufs=2, name="pw")
                nc.vector.max(pw[:, 0:8], pm[:])
                pm2 = smpool.tile([C, NCAND], f32, tag="pm2", name="pm2")
                nc.vector.match_replace(pm2[:], pw[:, 0:8], pm[:], 0.0)
                nc.vector.max(pw[:, 8:16], pm2[:])

                # decode winners: pair idx gp + value -> weights
                gidxf = smpool.tile([C, 16], f32, tag="gidxf", bufs=2,
                                    name="gidxf")
                nc.vector.tensor_scalar(
                    gidxf[:], pw[:], 1.0 / 1024.0, -0.5, Alu.mult, Alu.add
                )
                nc.vector.tensor_scalar(
                    gidxf[:], gidxf[:], MAGIC, MAGIC, Alu.add, Alu.subtract
                )
                vv = smpool.tile([C, 16], f32, tag="vv", name="vv")
                nc.vector.scalar_tensor_tensor(
                    vv[:], gidxf[:], -1024.0, pw[:], Alu.mult, Alu.add
                )
                nc.vector.tensor_scalar(
                    vv[:], vv[:], 1.0 / 128.0, -4.0, Alu.mult, Alu.add
                )
                expv = smpool.tile([C, 16], f32, tag="expv", name="expv")
                nc.scalar.activation(
                    expv[:], vv[:], mybir.ActivationFunctionType.Exp
                )
                wgt = smpool.tile([C, 16], f32, tag="wgt", name="wgt")
                nc.vector.scalar_tensor_tensor(
                    wgt[:], pw[:], 0.5, expv[:], Alu.is_ge, Alu.mult
                )
                den = smpool.tile([C, 1], f32, tag="den", name="den")
                nc.vector.tensor_reduce(
                    den[:], wgt[:], mybir.AxisListType.X, Alu.add
                )
                rden = smpool.tile([C, 1], f32, tag="rden", name="rden")
                nc.vector.reciprocal(rden[:], den[:])
                wn = smpool.tile([C, 16], f32, tag="wn", bufs=2, name="wn")
                nc.vector.tensor_scalar(
                    wn[:], wgt[:], rden[:], None, Alu.mult
                )

                gp_i = smpool.tile([C, 16], dt.int16, tag="gpi", bufs=2,
                                   name="gp_i")
                nc.vector.tensor_copy(gp_i[:], gidxf[:])

                # gather V pairs and score pairs (same idx lists)
                g_s = smpool.tile([C, 512], bf16, tag="G", bufs=2, name="g_s")
                nc.gpsimd.ap_gather(
                    g_s[:], st["vt_s"][:], gp_i[:],
                    channels=C, num_elems=NP, d=2, num_idxs=256,
                )
                eg_s = smpool.tile([C, 256], f32, tag="EG", bufs=2,
                                   name="eg_s")
                nc.gpsimd.ap_gather(
                    eg_s[:], st["s16e"][:], gp_i[:],
                    channels=C, num_elems=NP, d=1, num_idxs=256,
                )
                mg_s = smpool.tile([C, 256], f32, tag="MG", bufs=2,
                                   name="mg_s")
                nc.gpsimd.ap_gather(
                    mg_s[:], st["m_s"][:], gp_i[:],
                    channels=C, num_elems=NP, d=1, num_idxs=256,
                )
                st.update(g_s=g_s, eg_s=eg_s, mg_s=mg_s, wn=wn)

            def emit_late_tail(b, st):
                g_s, wn = st["g_s"], st["wn"]
                # per-row parity: even wins iff E == M (dd = E - M == 0)
                dd = smpool.tile([C, 256], f32, tag="dd", name="dd")
                nc.vector.tensor_sub(dd[:], st["eg_s"][:], st["mg_s"][:])
                nc.vector.tensor_mul(dd[:], dd[:], mmq_s[:])
                ddr = smpool.tile([C, 16], f32, tag="ddr", name="ddr")
                nc.vector.tensor_reduce(
                    ddr[:],
                    dd[:].rearrange("p (i q) -> p i q", q=NQ),
                    mybir.AxisListType.X,
                    Alu.add,
                )
                wnE = smpool.tile([C, 16], f32, tag="wnE", name="wnE")
                nc.vector.scalar_tensor_tensor(
                    wnE[:], ddr[:], 0.0, wn[:], Alu.is_ge, Alu.mult
                )
                wnO = smpool.tile([C, 16], f32, tag="wnO", name="wnO")
                nc.vector.tensor_sub(wnO[:], wn[:], wnE[:])

                # weights (parity-split) -> [(h,d), (i,q,r)] via headrep
                wEb = (
                    wnE[:].rearrange("p (i o) -> p i o", o=1)
                    .to_broadcast([C, NQ, 32])
                )
                wOb = (
                    wnO[:].rearrange("p (i o) -> p i o", o=1)
                    .to_broadcast([C, NQ, 32])
                )
                tmpE = smpool.tile([C, 512], f32, tag="tmpE", name="tmpE")
                nc.vector.tensor_mul(
                    tmpE[:].rearrange("p (i s) -> p i s", s=32),
                    wEb,
                    me_s[:].rearrange("p (i s) -> p i s", s=32),
                )
                wsc = smpool.tile([C, 512], f32, tag="wsc", name="wsc")
                nc.vector.tensor_mul(
                    wsc[:].rearrange("p (i s) -> p i s", s=32),
                    wOb,
                    mo_s[:].rearrange("p (i s) -> p i s", s=32),
                )
                nc.vector.tensor_add(wsc[:], wsc[:], tmpE[:])
                wb_ps = ps_v.tile([C, CHUNK], f32, tag="v", name="wb_ps")
                nc.tensor.matmul(wb_ps[:, 0:512], hrep_s[:], wsc[:])
                wb_s = smpool.tile([C, 512], bf16, tag="wb", name="wb_s")
                nc.scalar.copy(wb_s[:], wb_ps[:, 0:512])

                gw = smpool.tile([C, 512], f32, tag="gw", name="gw")
                nc.vector.tensor_mul(gw[:], g_s[:], wb_s[:])
                nc.vector.tensor_reduce(
                    pvt4_s[:, b::BPC],
                    gw[:].rearrange("p (i q r) -> p q i r", q=NQ, r=2),
                    mybir.AxisListType.XY,
                    Alu.add,
                )

            states = {}
            for b in range(BPC):
                states[b] = emit_chunks(b)
                if b == 0:
                    emit_const_dmas()
                emit_early_tail(b, states[b])
                if b >= 1:
                    emit_late_tail(b - 1, states[b - 1])
            emit_late_tail(BPC - 1, states[BPC - 1])

            # ---- final projections for all 4 batches ----
            o1_ps = ps_sel.tile([C, CHUNK], f32, tag="sel")
            for q in range(NQ):
                nc.tensor.matmul(
                    o1_ps[:, 0:BPC],
                    wjwp_s[:, q * C:(q + 1) * C],
                    pvt4_s[:, q * BPC:(q + 1) * BPC],
                    start=(q == 0),
                    stop=(q == NQ - 1),
                )
            o2_s = smpool.tile([C, BPC], f32, tag="o2")
            nc.vector.tensor_add(o2_s[:], o1_ps[:, 0:BPC], xT_s[:])
            o3_ps = ps_v.tile([C, CHUNK], f32, tag="v")
            nc.tensor.matmul(o3_ps[0:BPC, 0:C], o2_s[:], wp_s[:])
            o4_s = smpool.tile([BPC, C], f32, tag="o4")
            nc.vector.tensor_add(o4_s[:], o3_ps[0:BPC, 0:C], bp4_s[:])
            nc.sync.dma_start(out_d[:], o4_s[:])

    nc.compile()
    return nc


def _host_prep(inputs):
    x = np.asarray(inputs["x"], dtype=np.float32)              # [32, 1, 128]
    complement = np.asarray(inputs["complement"], np.float32)  # [32, 2047, 128]
    Wq = np.asarray(inputs["Wq"], np.float32)
    Wkv = np.asarray(inputs["Wkv"], np.float32)
    Wjw = np.asarray(inputs["Wjw"], np.float32)
    Wp = np.asarray(inputs["Wp"], np.float32)
    bp = np.asarray(inputs["bp"], np.float32)

    wkT = np.empty((C, 8 * C), np.float32)
    wv = np.empty((C, 8 * C), np.float32)
    for e in range(8):
        wkT[:, e * C:(e + 1) * C] = Wkv[:, e * 256: e * 256 + 128].T
        wv[:, e * C:(e + 1) * C] = Wkv[:, e * 256 + 128: e * 256 + 256]
    wv = wv.astype(np.float16)
    # host-side A_e[c,row] = Wk_e^T @ Qbd (0.25-scaled block-diag Q)
    qt_all = (x.reshape(B, C) @ Wq)                       # [B, 2048]
    a16_all = np.empty((B, C, 8 * C), np.float16)
    qbd = np.zeros((B, C, C), np.float32)
    for hh in range(H):
        for qq in range(NQ):
            qbd[:, hh * HD:(hh + 1) * HD, hh * NQ + qq] = (
                qt_all[:, qq * C + hh * HD: qq * C + (hh + 1) * HD] * 0.25
            )
    for e in range(8):
        blk = np.einsum(
            'cr,bcx->brx', wkT[:, e * C:(e + 1) * C], qbd
        )  # [B, row?, ...] -> A = wkT_e.T @ qbd per batch
        a16_all[:, :, e * C:(e + 1) * C] = blk.astype(np.float16)
    # Wjw rows are (h,q,d); per-q slice with rows (h,d)
    wjwp = (
        Wjw.reshape(H, NQ, HD, C).transpose(1, 0, 2, 3).reshape(NQ, C, C)
        .transpose(1, 0, 2).reshape(C, NQ * C)
    )
    bp4 = np.tile(bp.reshape(1, C), (BPC, 1)).astype(np.float32)
    hrep = np.kron(np.eye(H, dtype=np.float32), np.ones((HD, HD), np.float32))
    # cand slot -> chunk pair-offset (pairs per chunk = 512)
    choffrow = ((np.arange(NCAND) // 8) * (1024 * 1024)).astype(np.float32)
    choff = np.tile(choffrow.reshape(1, NCAND), (C, 1))
    s_idx = np.tile(np.arange(32).reshape(1, 1, 32), (C, NQ, 1))
    p_idx = (np.arange(C) % NQ).reshape(C, 1, 1)
    me = (s_idx == 2 * p_idx).astype(np.float32).reshape(C, 512)
    mo = (s_idx == 2 * p_idx + 1).astype(np.float32).reshape(C, 512)
    q_idx = np.tile(np.arange(NQ).reshape(1, 1, NQ), (C, NQ, 1))
    mmq = (q_idx == p_idx).astype(np.float32).reshape(C, 256)

    shared = dict(
        wv=np.ascontiguousarray(wv),
        wjwp=np.ascontiguousarray(wjwp),
        wp=np.ascontiguousarray(Wp),
        bp4=bp4,
        hrep=np.ascontiguousarray(hrep),
        choff=np.ascontiguousarray(choff),
        me=np.ascontiguousarray(me),
        mo=np.ascontiguousarray(mo),
        mmq=np.ascontiguousarray(mmq),
    )

    in_maps = []
    for core in range(CORES):
        bs = range(core * BPC, (core + 1) * BPC)
        comp = np.stack(
            [
                np.concatenate([x[b].reshape(1, C), complement[b]], axis=0)
                for b in bs
            ]
        ).astype(np.float32)
        compT = comp.transpose(0, 2, 1)          # [BPC, C, NC]
        comphT = compT.astype(np.float16)
        # S-side permutation: j lives at chunk ch = j // 1024 (within the
        # e-block column range), position: even j -> (j%1024)//2,
        # odd j -> 512 + (j%1024)//2.  comp column for score j is j % 2048
        # handled per e on device; here permute each 1024-col chunk of NC.
        comphP = comphT.reshape(BPC, C, NC // 1024, 512, 2)
        comphP = np.ascontiguousarray(
            comphP.transpose(0, 1, 2, 4, 3).reshape(BPC, C, NC)
        )
        xT = np.ascontiguousarray(x[list(bs)].reshape(BPC, C).T)
        m = dict(shared)
        m["comphT"] = np.ascontiguousarray(comphT)
        m["comphP"] = comphP
        m["a16"] = np.ascontiguousarray(a16_all[list(bs)])
        m["xT"] = xT
        in_maps.append(m)
    return in_maps


def kernel(**inputs):
    from concourse.bass_utils import run_bass_kernel_spmd

    if "prog" not in _prog_cache:
        _prog_cache["prog"] = _build_program()
    nc = _prog_cache["prog"]

    in_maps = _host_prep(inputs)
    res = run_bass_kernel_spmd(nc, in_maps, core_ids=list(range(CORES)))
    out = np.empty((B, 1, C), np.float32)
    for core in range(CORES):
        o = res.results[core]["out"]
        for i in range(BPC):
            out[core * BPC + i, 0, :] = o[i]
    return out


if __name__ == "__main__":
    d = np.load("/root/problem/inputs_cache.npz")
    inputs = {k: d[k] for k in d.files}
    got = kernel(**inputs)
    print("kernel output:", got.shape, got.dtype, np.abs(got).max())


# revision 27
# speedup vs baseline: 1.2085x; 1.0156x over previous
"""Trainium2 Bass kernel for nn_MultiHeadCrossAttention (B=32, Nc=2048, H=8, topk=12).

kernel(**inputs) takes FULL inputs, returns FULL output [32, 1, 128].
Batch is sharded 4-per-core across 8 NeuronCores (data parallel, no collectives).

v5 per-batch device algorithm (rows=(h,q) 128 wide, j = e*2048+nc in [0,16384)):
  single-term fp16 score matmul S-chunk[row,1024] -> PSUM
  scalar evacuates S -> SBUF fp16 (S16)
  gpsimd folds adjacent pairs: M[row, pair] = max(S16[2p], S16[2p+1])  [C,512]/chunk
  DVE max8 + find_index8 on M (half-width!) -> cand values + PAIR index directly
  exact top-12 of cand via max8/match_replace; pack (pair_idx*1024 + q10)
  decode -> winner pair indices gp + weights (exp/sum)
  gather V-pairs from VT (bf16) and score-pairs from S16 (fp16) with same idx
  parity mask from score-pair compare picks the winner half; weighted reduce
  out = (PV flat @ WjwP) + x;  out = out @ Wp + bp
  (rel err ~6.6e-3 on the fixed input: fp16 scores + fp16 fold ties + q10 weights)
"""

import sys
import numpy as np

for p in ("/opt/trn_rl_repo",):
    if p not in sys.path:
        sys.path.insert(0, p)

import ml_dtypes

B, CORES, BPC = 32, 8, 4
H, HD, NQ, TK, C, NC = 8, 16, 16, 12, 128, 2048
NJ = 8 * NC            # 16384
NP = NJ // 2           # 8192 pairs
CHUNK = 1024           # PSUM tile width (j)
SCH = 2048             # selection chunk (j): 1024 pairs
NSC = NJ // SCH        # 8 selection chunks
NCAND = NSC * 8        # 64
NEG = -1e30
MAGIC = 12582912.0     # 2**23 + 2**22: add/sub rounds fp32 to nearest int

_prog_cache = {}


def _build_program():
    import concourse.bass as bass
    import concourse.mybir as mybir
    import concourse.tile as tile
    from concourse import bacc
    from concourse import library_config

    dt = mybir.dt
    Alu = mybir.AluOpType
    f32, f16, bf16 = dt.float32, dt.float16, dt.bfloat16
    nc = bacc.Bacc("TRN2", target_bir_lowering=False)

    comphT_d = nc.dram_tensor("comphT", [BPC, C, NC], f16, kind="ExternalInput")
    comphP_d = nc.dram_tensor("comphP", [BPC, C, NC], f16, kind="ExternalInput")
    a16_d = nc.dram_tensor("a16", [BPC, C, 8 * C], f16, kind="ExternalInput")
    xT_d = nc.dram_tensor("xT", [C, BPC], f32, kind="ExternalInput")
    wv_d = nc.dram_tensor("wv", [C, 8 * C], f16, kind="ExternalInput")
    wjwp_d = nc.dram_tensor("wjwp", [C, NQ * C], f32, kind="ExternalInput")
    wp_d = nc.dram_tensor("wp", [C, C], f32, kind="ExternalInput")
    bp4_d = nc.dram_tensor("bp4", [BPC, C], f32, kind="ExternalInput")
    hrep_d = nc.dram_tensor("hrep", [C, C], f32, kind="ExternalInput")
    choff_d = nc.dram_tensor("choff", [C, NCAND], f32, kind="ExternalInput")
    me_d = nc.dram_tensor("me", [C, 512], f32, kind="ExternalInput")
    mo_d = nc.dram_tensor("mo", [C, 512], f32, kind="ExternalInput")
    mmq_d = nc.dram_tensor("mmq", [C, 256], f32, kind="ExternalInput")
    out_d = nc.dram_tensor("out", [BPC, C], f32, kind="ExternalOutput")

    with tile.TileContext(nc) as tc:
        nc.gpsimd.load_library(library_config.ap_gather)
        with (
            tc.tile_pool(name="weights", bufs=1) as wpool,
            tc.tile_pool(name="compt", bufs=2) as ctpool,
            tc.tile_pool(name="bigV", bufs=2) as vpool,
            tc.tile_pool(name="small", bufs=1) as smpool,
            tc.tile_pool(name="ps_sel", bufs=2, space="PSUM") as ps_sel,
            tc.tile_pool(name="ps_v", bufs=2, space="PSUM") as ps_v,
        ):
            # ---- chunk-phase weights first; the rest stream later ----
            wv_s = wpool.tile([C, 8 * C], f16)
            nc.sync.dma_start(wv_s[:], wv_d[:])
            hrep_s = wpool.tile([C, C], f32)
            nc.sync.dma_start(hrep_s[:], hrep_d[:])
            choff_s = wpool.tile([C, NCAND], f32)
            me_s = wpool.tile([C, 512], f32)
            mo_s = wpool.tile([C, 512], f32)
            mmq_s = wpool.tile([C, 256], f32)
            wjwp_s = wpool.tile([C, NQ * C], f32)
            wp_s = wpool.tile([C, C], f32)
            bp4_s = wpool.tile([BPC, C], f32)
            xT_s = wpool.tile([C, BPC], f32)

            def emit_const_dmas():
                nc.sync.dma_start(choff_s[:], choff_d[:])
                nc.sync.dma_start(me_s[:], me_d[:])
                nc.sync.dma_start(mo_s[:], mo_d[:])
                nc.sync.dma_start(mmq_s[:], mmq_d[:])
                nc.sync.dma_start(wjwp_s[:], wjwp_d[:])
                nc.sync.dma_start(wp_s[:], wp_d[:])
                nc.sync.dma_start(bp4_s[:], bp4_d[:])
                nc.sync.dma_start(xT_s[:], xT_d[:])

            pvt4_s = wpool.tile([C, NQ * BPC], f32)   # [(h,d), (q,b)]

            def emit_chunks(b):
                st = {}
                c16p = ctpool.tile([C, NC], f16, tag="c16p", name="c16p")
                nc.sync.dma_start(c16p[:], comphP_d[b])
                a16h = smpool.tile([C, 8 * C], f16, tag="a16h", bufs=2,
                                   name="a16h")
                nc.sync.dma_start(a16h[:], a16_d[b])
                c16h = ctpool.tile([C, NC], f16, tag="c16h", name="c16h")
                nc.sync.dma_start(c16h[:], comphT_d[b])
                cand_s = smpool.tile([C, NCAND], f32, tag="cand", bufs=2,
                                     name="cand_s")
                li_s = smpool.tile([C, NCAND], dt.uint16, tag="li", bufs=2,
                                   name="li_s")
                vt_s = vpool.tile([C, NJ], bf16, tag="VT", name="vt_s")
                s16e = vpool.tile([C, NP], f32, tag="S16E", bufs=1,
                                  name="s16e")
                m_s = vpool.tile([C, NP], f32, tag="M", bufs=1, name="m_s")
                for e in range(8):
                    s_list = []
                    for half in range(2):
                        s_ps = ps_sel.tile([C, CHUNK], f32, tag="sel",
                                           name="s_ps")
                        s_list.append(s_ps)
                        for n in range(2):
                            col = half * 1024 + n * 512
                            nc.tensor.matmul(
                                s_ps[:, n * 512:(n + 1) * 512],
                                a16h[:, e * C:(e + 1) * C],
                                c16p[:, col:col + 512],
                            )
                    # selection path first (scalar ec feeds DVE fold)
                    for half in range(2):
                        ch = e * 2 + half
                        s_ps = s_list[half]
                        ec = s16e[:, ch * 512:(ch + 1) * 512]
                        nc.scalar.copy(ec, s_ps[:, 0:512])
                        nc.vector.tensor_max(
                            m_s[:, ch * 512:(ch + 1) * 512],
                            ec,
                            s_ps[:, 512:1024],
                        )
                    nc.vector.max(
                        cand_s[:, e * 8:(e + 1) * 8],
                        m_s[:, e * CHUNK:(e + 1) * CHUNK],
                    )
                    nc.vector.max_index(
                        li_s[:, e * 8:(e + 1) * 8],
                        cand_s[:, e * 8:(e + 1) * 8],
                        m_s[:, e * CHUNK:(e + 1) * CHUNK],
                    )
                    # V path trails (nothing reads VT until the gather)
                    for half in range(2):
                        v_ps = ps_v.tile([C, CHUNK], f32, tag="v", name="v_ps")
                        for n in range(2):
                            col = half * 1024 + n * 512
                            nc.tensor.matmul(
                                v_ps[:, n * 512:(n + 1) * 512],
                                wv_s[:, e * C:(e + 1) * C],
                                c16h[:, col:col + 512],
                            )
                        nc.scalar.copy(
                            vt_s[:, e * NC + half * 1024:
                                 e * NC + (half + 1) * 1024],
                            v_ps[:],
                        )
                st.update(cand_s=cand_s, li_s=li_s, vt_s=vt_s, s16e=s16e,
                          m_s=m_s)
                return st

            def emit_early_tail(b, st):
                cand_s, li_s = st["cand_s"], st["li_s"]
                # exact top-12 marking on cand
                t8a = smpool.tile([C, 8], f32, tag="t8a", name="t8a")
                nc.vector.max(t8a[:], cand_s[:])
                c2 = smpool.tile([C, NCAND], f32, tag="c2", name="c2")
                nc.vector.match_replace(c2[:], t8a[:], cand_s[:], NEG)
                t8b = smpool.tile([C, 8], f32, tag="t8b", name="t8b")
                nc.vector.max(t8b[:], c2[:])
                nx4 = smpool.tile([C, 8], f32, tag="nx4", name="nx4")
                nc.vector.memset(nx4[:], 1e30)
                nc.vector.tensor_copy(nx4[:, 0:4], t8b[:, 0:4])
                rr = smpool.tile([C, NCAND], f32, tag="rr", name="rr")
                nc.vector.match_replace(rr[:], nx4[:], c2[:], NEG)
                mask12 = smpool.tile([C, NCAND], f32, tag="mask12",
                                     name="mask12")
                nc.vector.tensor_scalar(
                    mask12[:], rr[:], -1e29, None, Alu.is_le
                )

                # pack pair_idx*1024 + q10(value); mask; extract
                lif = smpool.tile([C, NCAND], f32, tag="lif", name="lif")
                nc.vector.tensor_copy(lif[:], li_s[:])
                gfl = smpool.tile([C, NCAND], f32, tag="gfl", name="gfl")
                nc.vector.scalar_tensor_tensor(
                    gfl[:], lif[:], 1024.0, choff_s[:], Alu.mult, Alu.add
                )
                q10 = smpool.tile([C, NCAND], f32, tag="q10", name="q10")
                nc.vector.tensor_scalar(
                    q10[:], cand_s[:], 4.0, 128.0, Alu.add, Alu.mult
                )
                nc.vector.tensor_scalar(
                    q10[:], q10[:], 1023.0, 1.0, Alu.min, Alu.max
                )
                pm = smpool.tile([C, NCAND], f32, tag="pm", name="pm")
                nc.vector.tensor_add(pm[:], gfl[:], q10[:])
                nc.vector.tensor_mul(pm[:], pm[:], mask12[:])

                pw = smpool.tile([C, 16], f32, tag="pw", bufs=2, name="pw")
                nc.vector.max(pw[:, 0:8], pm[:])
                pm2 = smpool.tile([C, NCAND], f32, tag="pm2", name="pm2")
                nc.vector.match_replace(pm2[:], pw[:, 0:8], pm[:], 0.0)
                nc.vector.max(pw[:, 8:16], pm2[:])

                # decode winners: pair idx gp + value -> weights
                gidxf = smpool.tile([C, 16], f32, tag="gidxf", bufs=2,
                                    name="gidxf")
                nc.vector.tensor_scalar(
                    gidxf[:], pw[:], 1.0 / 1024.0, -0.5, Alu.mult, Alu.add
                )
                nc.vector.tensor_scalar(
                    gidxf[:], gidxf[:], MAGIC, MAGIC, Alu.add, Alu.subtract
                )
                vv = smpool.tile([C, 16], f32, tag="vv", name="vv")
                nc.vector.scalar_tensor_tensor(
                    vv[:], gidxf[:], -1024.0, pw[:], Alu.mult, Alu.add
                )
                nc.vector.tensor_scalar(
                    vv[:], vv[:], 1.0 / 128.0, -4.0, Alu.mult, Alu.add
                )
                expv = smpool.tile([C, 16], f32, tag="expv", name="expv")
                nc.scalar.activation(
                    expv[:], vv[:], mybir.ActivationFunctionType.Exp
                )
                wgt = smpool.tile([C, 16], f32, tag="wgt", name="wgt")
                nc.vector.scalar_tensor_tensor(
                    wgt[:], pw[:], 0.5, expv[:], Alu.is_ge, Alu.mult
                )
                den = smpool.tile([C, 1], f32, tag="den", name="den")
                nc.vector.tensor_reduce(
                    den[:], wgt[:], mybir.AxisListType.X, Alu.add
                )
                rden = smpool.tile([C, 1], f32, tag="rden", name="rden")
                nc.vector.reciprocal(rden[:], den[:])
                wn = smpool.tile([C, 16], f32, tag="wn", bufs=2, name="wn")
                nc.vector.tensor_scalar(
                    wn[:], wgt[:], rden[:], None, Alu.mult
                )

                gp_i = smpool.tile([C, 16], dt.int16, tag="gpi", bufs=2,
                                   name="gp_i")
                nc.vector.tensor_copy(gp_i[:], gidxf[:])

                # gather V pairs and score pairs (same idx lists)
                g_s = smpool.tile([C, 512], bf16, tag="G", bufs=2, name="g_s")
                nc.gpsimd.ap_gather(
                    g_s[:], st["vt_s"][:], gp_i[:],
                    channels=C, num_elems=NP, d=2, num_idxs=256,
                )
                eg_s = smpool.tile([C, 256], f32, tag="EG", bufs=2,
                                   name="eg_s")
                nc.gpsimd.ap_gather(
                    eg_s[:], st["s16e"][:], gp_i[:],
                    channels=C, num_elems=NP, d=1, num_idxs=256,
                )
                mg_s = smpool.tile([C, 256], f32, tag="MG", bufs=2,
                                   name="mg_s")
                nc.gpsimd.ap_gather(
                    mg_s[:], st["m_s"][:], gp_i[:],
                    channels=C, num_elems=NP, d=1, num_idxs=256,
                )
                st.update(g_s=g_s, eg_s=eg_s, mg_s=mg_s, wn=wn)

            def emit_late_tail(b, st):
                g_s, wn = st["g_s"], st["wn"]
                # per-row parity: even wins iff E == M (dd = E - M == 0)
                dd = smpool.tile([C, 256], f32, tag="dd", name="dd")
                nc.vector.tensor_sub(dd[:], st["eg_s"][:], st["mg_s"][:])
                nc.vector.tensor_mul(dd[:], dd[:], mmq_s[:])
                ddr = smpool.tile([C, 16], f32, tag="ddr", name="ddr")
                nc.vector.tensor_reduce(
                    ddr[:],
                    dd[:].rearrange("p (i q) -> p i q", q=NQ),
                    mybir.AxisListType.X,
                    Alu.add,
                )
                wnE = smpool.tile([C, 16], f32, tag="wnE", name="wnE")
                nc.vector.scalar_tensor_tensor(
                    wnE[:], ddr[:], 0.0, wn[:], Alu.is_ge, Alu.mult
                )
                wnO = smpool.tile([C, 16], f32, tag="wnO", name="wnO")
                nc.vector.tensor_sub(wnO[:], wn[:], wnE[:])

                # weights (parity-split) -> [(h,d), (i,q,r)] via headrep
                wEb = (
                    wnE[:].rearrange("p (i o) -> p i o", o=1)
                    .to_broadcast([C, NQ, 32])
                )
                wOb = (
                    wnO[:].rearrange("p (i o) -> p i o", o=1)
                    .to_broadcast([C, NQ, 32])
                )
                tmpE = smpool.tile([C, 512], f32, tag="tmpE", name="tmpE")
                nc.vector.tensor_mul(
                    tmpE[:].rearrange("p (i s) -> p i s", s=32),
                    wEb,
                    me_s[:].rearrange("p (i s) -> p i s", s=32),
                )
                wsc = smpool.tile([C, 512], f32, tag="wsc", name="wsc")
                nc.vector.tensor_mul(
                    wsc[:].rearrange("p (i s) -> p i s", s=32),
                    wOb,
                    mo_s[:].rearrange("p (i s) -> p i s", s=32),
                )
                nc.vector.tensor_add(wsc[:], wsc[:], tmpE[:])
                wb_ps = ps_v.tile([C, CHUNK], f32, tag="v", name="wb_ps")
                nc.tensor.matmul(wb_ps[:, 0:512], hrep_s[:], wsc[:])
                wb_s = smpool.tile([C, 512], bf16, tag="wb", name="wb_s")
                nc.scalar.copy(wb_s[:], wb_ps[:, 0:512])

                gw = smpool.tile([C, 512], f32, tag="gw", name="gw")
                nc.vector.tensor_mul(gw[:], g_s[:], wb_s[:])
                nc.vector.tensor_reduce(
                    pvt4_s[:, b::BPC],
                    gw[:].rearrange("p (i q r) -> p q i r", q=NQ, r=2),
                    mybir.AxisListType.XY,
                    Alu.add,
                )

            states = {}
            for b in range(BPC):
                states[b] = emit_chunks(b)
                if b == 0:
                    emit_const_dmas()
                emit_early_tail(b, states[b])
                if b >= 1:
                    emit_late_tail(b - 1, states[b - 1])
            emit_late_tail(BPC - 1, states[BPC - 1])

            # ---- final projections for all 4 batches ----
            o1_ps = ps_sel.tile([C, CHUNK], f32, tag="sel")
            for q in range(NQ):
                nc.tensor.matmul(
                    o1_ps[:, 0:BPC],
                    wjwp_s[:, q * C:(q + 1) * C],
                    pvt4_s[:, q * BPC:(q + 1) * BPC],
                    start=(q == 0),
                    stop=(q == NQ - 1),
                )
            o2_s = smpool.tile([C, BPC], f32, tag="o2")
            nc.vector.tensor_add(o2_s[:], o1_ps[:, 0:BPC], xT_s[:])
            o3_ps = ps_v.tile([C, CHUNK], f32, tag="v")
            nc.tensor.matmul(o3_ps[0:BPC, 0:C], o2_s[:], wp_s[:])
            o4_s = smpool.tile([BPC, C], f32, tag="o4")
            nc.vector.tensor_add(o4_s[:], o3_ps[0:BPC, 0:C], bp4_s[:])
            nc.sync.dma_start(out_d[:], o4_s[:])

    nc.compile()
    return nc


def _host_prep(inputs):
    x = np.asarray(inputs["x"], dtype=np.float32)              # [32, 1, 128]
    complement = np.asarray(inputs["complement"], np.float32)  # [32, 2047, 128]
    Wq = np.asarray(inputs["Wq"], np.float32)
    Wkv = np.asarray(inputs["Wkv"], np.float32)
    Wjw = np.asarray(inputs["Wjw"], np.float32)
    Wp = np.asarray(inputs["Wp"], np.float32)
    bp = np.asarray(inputs["bp"], np.float32)

    wkT = np.empty((C, 8 * C), np.float32)
    wv = np.empty((C, 8 * C), np.float32)
    for e in range(8):
        wkT[:, e * C:(e + 1) * C] = Wkv[:, e * 256: e * 256 + 128].T
        wv[:, e * C:(e + 1) * C] = Wkv[:, e * 256 + 128: e * 256 + 256]
    wv = wv.astype(np.float16)
    # host-side A_e[c,row] = Wk_e^T @ Qbd (0.25-scaled block-diag Q)
    qt_all = (x.reshape(B, C) @ Wq)                       # [B, 2048]
    a16_all = np.empty((B, C, 8 * C), np.float16)
    qbd = np.zeros((B, C, C), np.float32)
    for hh in range(H):
        for qq in range(NQ):
            qbd[:, hh * HD:(hh + 1) * HD, hh * NQ + qq] = (
                qt_all[:, qq * C + hh * HD: qq * C + (hh + 1) * HD] * 0.25
            )
    for e in range(8):
        blk = np.einsum(
            'cr,bcx->brx', wkT[:, e * C:(e + 1) * C], qbd
        )  # [B, row?, ...] -> A = wkT_e.T @ qbd per batch
        a16_all[:, :, e * C:(e + 1) * C] = blk.astype(np.float16)
    # Wjw rows are (h,q,d); per-q slice with rows (h,d)
    wjwp = (
        Wjw.reshape(H, NQ, HD, C).transpose(1, 0, 2, 3).reshape(NQ, C, C)
        .transpose(1, 0, 2).reshape(C, NQ * C)
    )
    bp4 = np.tile(bp.reshape(1, C), (BPC, 1)).astype(np.float32)
    hrep = np.kron(np.eye(H, dtype=np.float32), np.ones((HD, HD), np.float32))
    # cand slot -> chunk pair-offset (pairs per chunk = 512)
    choffrow = ((np.arange(NCAND) // 8) * (1024 * 1024)).astype(np.float32)
    choff = np.tile(choffrow.reshape(1, NCAND), (C, 1))
    s_idx = np.tile(np.arange(32).reshape(1, 1, 32), (C, NQ, 1))
    p_idx = (np.arange(C) % NQ).reshape(C, 1, 1)
    me = (s_idx == 2 * p_idx).astype(np.float32).reshape(C, 512)
    mo = (s_idx == 2 * p_idx + 1).astype(np.float32).reshape(C, 512)
    q_idx = np.tile(np.arange(NQ).reshape(1, 1, NQ), (C, NQ, 1))
    mmq = (q_idx == p_idx).astype(np.float32).reshape(C, 256)

    shared = dict(
        wv=np.ascontiguousarray(wv),
        wjwp=np.ascontiguousarray(wjwp),
        wp=np.ascontiguousarray(Wp),
        bp4=bp4,
        hrep=np.ascontiguousarray(hrep),
        choff=np.ascontiguousarray(choff),
        me=np.ascontiguousarray(me),
        mo=np.ascontiguousarray(mo),
        mmq=np.ascontiguousarray(mmq),
    )

    in_maps = []
    for core in range(CORES):
        bs = range(core * BPC, (core + 1) * BPC)
        comp = np.stack(
            [
                np.concatenate([x[b].reshape(1, C), complement[b]], axis=0)
                for b in bs
            ]
        ).astype(np.float32)
        compT = comp.transpose(0, 2, 1)          # [BPC, C, NC]
        comphT = compT.astype(np.float16)
        # S-side permutation: j lives at chunk ch = j // 1024 (within the
        # e-block column range), position: even j -> (j%1024)//2,
        # odd j -> 512 + (j%1024)//2.  comp column for score j is j % 2048
        # handled per e on device; here permute each 1024-col chunk of NC.
        comphP = comphT.reshape(BPC, C, NC // 1024, 512, 2)
        comphP = np.ascontiguousarray(
            comphP.transpose(0, 1, 2, 4, 3).reshape(BPC, C, NC)
        )
        xT = np.ascontiguousarray(x[list(bs)].reshape(BPC, C).T)
        m = dict(shared)
        m["comphT"] = np.ascontiguousarray(comphT)
        m["comphP"] = comphP
        m["a16"] = np.ascontiguousarray(a16_all[list(bs)])
        m["xT"] = xT
        in_maps.append(m)
    return in_maps


def kernel(**inputs):
    from concourse.bass_utils import run_bass_kernel_spmd

    if "prog" not in _prog_cache:
        _prog_cache["prog"] = _build_program()
    nc = _prog_cache["prog"]

    in_maps = _host_prep(inputs)
    res = run_bass_kernel_spmd(nc, in_maps, core_ids=list(range(CORES)))
    out = np.empty((B, 1, C), np.float32)
    for core in range(CORES):
        o = res.results[core]["out"]
        for i in range(BPC):
            out[core * BPC + i, 0, :] = o[i]
    return out


if __name__ == "__main__":
    d = np.load("/root/problem/inputs_cache.npz")
    inputs = {k: d[k] for k in d.files}
    got = kernel(**inputs)
    print("kernel output:", got.shape, got.dtype, np.abs(got).max())
